# revision 7
# baseline (speedup 1.0000x reference)
"""Depth-to-space (pixel shuffle / DUC) kernel for Trainium2.

Full op: x[16, 1216, 32, 32] f32 -> out[16, 19, 304, 304] f32 where
  out[b, c, i*8+r1, j*8+r2] = x[b, c*64 + r1*8 + r2, i, j]
and out is zero-padded from 256x256 up to 304x304 (bottom/right).

Sharding: pure data-parallel over batch, 2 examples per core on 8 cores.

The op is pure data movement.  HW measurements fit a per-descriptor
cost of max(bytes/22.5ns, ~41ns) per SDMA engine, so every descriptor
must be >= 1KB to be byte-bound (512B descriptors waste half the
engine).  With the 3-dims-per-side DMA AP limit, 1KB load runs force
4 partitions per image (i = ihh*8 + il3, run = (il3, j) = 1KB):

  SBUF tile A [128, 16384] f32 covers up to 32 images (one per slot):
    partition p = ihh*32 + s        (ihh = i>>3, 4 partitions/image)
    free       = ch*256 + il3*32 + j     (ch = r1*8+r2, il3 = i&7)
  - Load: one 3-dim DMA per image, dims (ihh, ch, run), 1KB runs,
    DRAM reads fully sequential.
  - Shuffle: rows y = ihh*64 + il3*8 + r1; 8 DVE/Act strided
    sub-copies per set (one per il3) produce B halves with
    free = u*256 + j*8 + r2 (u = il3l*8+r1, row-halves h = il3>>2).
  - Store: one 3-dim DMA per (image, half), dims (ihh, u, run), 1KB.

An image's 4 partitions land on SDMA engines {g, g+4, g+8, g+12}
where g = slot>>3 (engine = partition>>3), so images are spread over
4 slot-groups and each group issues from its own DMA queue (sync /
scalar / vector HWDGE + gpsimd SWDGE) -- 4 concurrent queues hitting
disjoint engine quads keep all 16 SDMA engines busy.

Per core: 2 sets of 19 images; 38 loads + 76 half-stores + 16 copies.
SBUF: A bufs=2 (64KB/part) + B-half bufs=2 (32KB/part) = 192KB/part.
Zero padding relies on ExternalOutput buffers being pre-zeroed by the
runner (both the native path and the PJRT/axon path guarantee this).
"""

import sys

if "/opt/trn_rl_repo" not in sys.path:
    sys.path.insert(0, "/opt/trn_rl_repo")

import numpy as np

B, CLASSES, R, H, W, OUT = 16, 19, 8, 32, 32, 304
HR = H * R  # 256
N_CORES = 8
BPC = B // N_CORES  # batches per core = 2

_NC_CACHE = {}

# 19 slots spread over the 4 slot-groups; the two sets rotate which
# group gets only 4 images so per-engine work balances across the body.
_SLOTS_A = [0, 1, 2, 3, 4, 8, 9, 10, 11, 12, 16, 17, 18, 19, 20, 24, 25, 26, 27]
_SLOTS_B = [0, 1, 2, 3, 8, 9, 10, 11, 12, 16, 17, 18, 19, 20, 24, 25, 26, 27, 28]


def build_nc(
    bpc=BPC,
    classes=CLASSES,
    num_devices=N_CORES,
    repeats=1,
    loop_repeats=1,
):
    import concourse.bacc as bacc
    import concourse.mybir as mybir
    from concourse.tile import TileContext

    f32 = mybir.dt.float32
    nc = bacc.Bacc(
        "TRN2", target_bir_lowering=False, debug=False, num_devices=num_devices
    )
    x = nc.declare_dram_parameter("x", [bpc, classes * R * R, H, W], f32, isOutput=False)
    out = nc.declare_dram_parameter("out", [bpc, classes, OUT, OUT], f32, isOutput=True)

    n_img = bpc * classes  # 38
    imgs = list(range(n_img))
    half = (n_img + 1) // 2
    sets = [
        (imgs[:half], _SLOTS_A[:half]),
        (imgs[half:], _SLOTS_B[: n_img - half]),
    ]

    def dma_eng(slot):
        # Two HWDGE queues, two slot-groups each (group g's partitions hit
        # engine quad {g+4k}).  Each queue keeps 4 DMAs in flight via the
        # DMAHW sub-queue round-robin, so alternating its two groups in
        # emission order keeps both engine quads busy concurrently.
        return nc.sync if (slot >> 3) < 2 else nc.scalar

    with TileContext(nc) as tc:
        with (
            tc.tile_pool(name="raw", bufs=2) as raw_pool,
            tc.tile_pool(name="row", bufs=2) as row_pool,
        ):
          def _body():
            sinfo = []
            for tl, slots in sets:
                A = raw_pool.tile([128, 16384], f32)
                sinfo.append((tl, slots, A))

            def interleaved(tl, slots):
                # emit order (slot&7, slot>>3): each queue alternates its
                # two slot-groups so consecutive DMAs hit different quads
                return sorted(zip(tl, slots), key=lambda p: (p[1] & 7, p[1] >> 3))

            def emit_loads(si):
                tl, slots, A = sinfo[si]
                Av = A[:].rearrange(
                    "(ihh s) (ch il3 j) -> s ihh ch il3 j", s=32, ch=64, il3=8
                )
                for m, slot in interleaved(tl, slots):
                    b, c = divmod(m, classes)
                    src = x[b, c * 64 : (c + 1) * 64].rearrange(
                        "ch (ihh il3) j -> ihh ch il3 j", ihh=4
                    )
                    dma_eng(slot).dma_start(out=Av[slot], in_=src)

            def emit_half(si, h):
                """Copy il3-half h of set si into a fresh B-half tile and
                store it; returns nothing (emission order = program order)."""
                tl, slots, A = sinfo[si]
                Bh = row_pool.tile([128, 8192], f32)
                s_full = A[:].rearrange(
                    "p (r1 r2 il3 j) -> p il3 r1 j r2", r1=8, r2=8, il3=8
                )
                d_full = Bh[:].rearrange(
                    "p (il3l r1 j r2) -> p il3l r1 j r2", il3l=4, r1=8, j=32
                )
                for k in range(4):
                    # all copies on DVE: both HWDGE queues are busy issuing
                    nc.vector.tensor_copy(d_full[:, k], s_full[:, h * 4 + k])
                Bv = Bh[:].rearrange("(ihh s) (u xx) -> s ihh u xx", s=32, u=32)
                for m, slot in interleaved(tl, slots):
                    b, c = divmod(m, classes)
                    dst = out[b, c, 0:HR, 0:HR].rearrange(
                        "(ihh hb u) xx -> hb ihh u xx", ihh=4, hb=2
                    )[h]
                    dma_eng(slot).dma_start(out=dst, in_=Bv[slot])

            emit_loads(0)
            emit_loads(1)
            emit_half(0, 0)
            emit_half(0, 1)
            emit_half(1, 0)
            emit_half(1, 1)

          if loop_repeats > 1:
              # measurement-only: on-device loop to amortize dispatch noise
              with tc.For_i(0, loop_repeats, 1):
                  _body()
          else:
              for _rep in range(repeats):
                  _body()
    nc.compile()
    return nc


def _get_nc():
    key = "main"
    if key not in _NC_CACHE:
        _NC_CACHE[key] = build_nc()
    return _NC_CACHE[key]


def kernel(x: np.ndarray) -> np.ndarray:
    from concourse.bass_utils import run_bass_kernel_spmd

    x = np.ascontiguousarray(x, dtype=np.float32)
    assert x.shape == (B, CLASSES * R * R, H, W), x.shape
    nc = _get_nc()
    in_maps = [{"x": x[k * BPC : (k + 1) * BPC]} for k in range(N_CORES)]
    res = run_bass_kernel_spmd(nc, in_maps, list(range(N_CORES)))
    return np.concatenate([res.results[k]["out"] for k in range(N_CORES)], axis=0)


# revision 8
# speedup vs baseline: 2.0167x; 2.0167x over previous
"""Depth-to-space (pixel shuffle / DUC) kernel for Trainium2.

Full op: x[16, 1216, 32, 32] f32 -> out[16, 19, 304, 304] f32 where
  out[b, c, i*8+r1, j*8+r2] = x[b, c*64 + r1*8 + r2, i, j]
and out is zero-padded from 256x256 up to 304x304 (bottom/right).

Sharding: pure data-parallel over batch, 2 examples per core on 8 cores.

The op is pure data movement.  HW measurements fit a queue-serial DMA
model: each HWDGE queue (sync=SP, scalar=Act) executes one DMA at a
time on the engine span of that DMA (SDMA engine = partition>>3), at
per-descriptor cost max(bytes/22.5ns, ~41ns) per engine.  The layout
keeps every DMA on 8 partitions spread over one 8-engine octet, with
the two queues serving disjoint octets concurrently:

  SBUF tile A [128, 8192] f32 covers 16 images (one per slot s):
    partition p = ihh*16 + s      (ihh = i>>2, 8 partitions per image)
    free       = ch*128 + il2*32 + j   (ch = r1*8+r2, il2 = i&3)
  - Load: one 3-dim DMA per image, dims (ihh, ch, run); runs (il2, j)
    = 512B contiguous on both sides; DRAM reads fully sequential.
  - Shuffle: rows y = i*8+r1 = ihh*32 + (il2*8+r1); one DVE strided
    copy per (tile, il2) produces free = u*256 + j*8 + r2 (u = il2*8+r1).
  - Store: one 3-dim DMA per image, dims (ihh, u, run), 1KB runs.

Octet = slot>>3.  queue_mode 'parity' (default): each queue owns one
octet (sync: slots<8, scalar: slots>=8).  queue_mode 'split': sync
issues all loads and scalar all stores, alternating octets, to probe
DMAHW sub-queue transfer concurrency.

Per core: 38 loads + 38 stores + 12 DVE copies; measured 77.6us
(= the zero-bubble queue-serial floor for this topology).
Zero padding relies on ExternalOutput buffers being pre-zeroed by the
runner (both the native path and the PJRT/axon path guarantee this).
"""

import sys

if "/opt/trn_rl_repo" not in sys.path:
    sys.path.insert(0, "/opt/trn_rl_repo")

import numpy as np

B, CLASSES, R, H, W, OUT = 16, 19, 8, 32, 32, 304
HR = H * R  # 256
N_CORES = 8
BPC = B // N_CORES  # batches per core = 2

_NC_CACHE = {}


def build_nc(
    bpc=BPC,
    classes=CLASSES,
    num_devices=N_CORES,
    repeats=1,
    loop_repeats=1,
    bufs=2,
    imgs_per_tile=16,
    queue_mode="parity",
):
    import concourse.bacc as bacc
    import concourse.mybir as mybir
    from concourse.tile import TileContext

    f32 = mybir.dt.float32
    nc = bacc.Bacc(
        "TRN2", target_bir_lowering=False, debug=False, num_devices=num_devices
    )
    x = nc.declare_dram_parameter("x", [bpc, classes * R * R, H, W], f32, isOutput=False)
    out = nc.declare_dram_parameter("out", [bpc, classes, OUT, OUT], f32, isOutput=True)

    n_img = bpc * classes  # 38
    FREE = 64 * 128  # 8192 floats = 32KB per partition

    # Plan tiles: list of [(img_idx, slot), ...].  Slots pick partitions
    # p = ihh*16 + slot; slot>=8 lands on the odd SDMA engine octet, so a
    # ragged last tile splits its images between the two parity halves.
    def tiles_plan():
        tiles = []
        idx = 0
        while idx < n_img:
            cnt = min(imgs_per_tile, n_img - idx)
            if cnt == imgs_per_tile:
                slots = list(range(imgs_per_tile))
            else:
                lo = (cnt + 1) // 2
                slots = list(range(lo)) + list(range(8, 8 + cnt - lo))
            tiles.append([(idx + k, slots[k]) for k in range(cnt)])
            idx += cnt
        return tiles

    tiles = tiles_plan()

    with TileContext(nc) as tc:
        with (
            tc.tile_pool(name="raw", bufs=bufs) as raw_pool,
            tc.tile_pool(name="row", bufs=bufs) as row_pool,
        ):
          def _body():
            tinfo = []
            for tl in tiles:
                A = raw_pool.tile([128, FREE], f32)
                Bt = row_pool.tile([128, FREE], f32)
                tinfo.append((tl, A, Bt))

            def eng_for(slot, is_load):
                if queue_mode == "parity":
                    # octet = slot>>3: one queue per octet
                    return nc.sync if slot < 8 else nc.scalar
                # 'split': loads on sync, stores on scalar
                return nc.sync if is_load else nc.scalar

            def order(tl):
                if queue_mode == "parity":
                    return tl
                # 'split': alternate octets so consecutive DMAs on a
                # queue hit different engine octets
                return sorted(tl, key=lambda p: (p[1] & 7, p[1] >> 3))

            def emit_loads(t):
                tl, A, _ = tinfo[t]
                # dst view: (s, ihh, ch, il2, j); SBUF DMA APs need the
                # partition dim (ihh, step 16) outermost, so both sides
                # enumerate (ihh, ch, run).
                Av = A[:].rearrange(
                    "(ihh s) (ch il2 j) -> s ihh ch il2 j", s=16, ch=64, il2=4
                )
                for m, slot in order(tl):
                    b, c = divmod(m, classes)
                    src = x[b, c * 64 : (c + 1) * 64].rearrange(
                        "ch (ihh il2) j -> ihh ch il2 j", il2=4
                    )
                    eng_for(slot, True).dma_start(out=Av[slot], in_=src)

            def emit_copy(t):
                _, A, Bt = tinfo[t]
                # B[p, il2*2048 + r1*256 + j*8 + r2] = A[p, (r1*8+r2)*128 + il2*32 + j]
                s_full = A[:].rearrange(
                    "p (r1 r2 il2 j) -> p il2 r1 j r2", r1=8, r2=8, il2=4
                )
                d_full = Bt[:].rearrange(
                    "p (il2 r1 j r2) -> p il2 r1 j r2", il2=4, r1=8, r2=8
                )
                for il2 in range(4):
                    nc.vector.tensor_copy(d_full[:, il2], s_full[:, il2])

            def emit_stores(t):
                tl, _, Bt = tinfo[t]
                Bv = Bt[:].rearrange("(ihh s) (u xx) -> s ihh u xx", s=16, u=32)
                for m, slot in order(tl):
                    b, c = divmod(m, classes)
                    dst = out[b, c, 0:HR, 0:HR].rearrange("(ihh u) xx -> ihh u xx", u=32)
                    eng_for(slot, False).dma_start(out=dst, in_=Bv[slot])

            nt = len(tinfo)
            for t in range(min(2, nt)):
                emit_loads(t)
            for t in range(nt):
                emit_copy(t)
                if t + 2 < nt:
                    emit_loads(t + 2)
                emit_stores(t)

          if loop_repeats > 1:
              # measurement-only: on-device loop to amortize dispatch noise
              with tc.For_i(0, loop_repeats, 1):
                  _body()
          else:
              for _rep in range(repeats):
                  _body()
    nc.compile()
    return nc


def _get_nc():
    key = "main"
    if key not in _NC_CACHE:
        _NC_CACHE[key] = build_nc()
    return _NC_CACHE[key]


def kernel(x: np.ndarray) -> np.ndarray:
    from concourse.bass_utils import run_bass_kernel_spmd

    x = np.ascontiguousarray(x, dtype=np.float32)
    assert x.shape == (B, CLASSES * R * R, H, W), x.shape
    nc = _get_nc()
    in_maps = [{"x": x[k * BPC : (k + 1) * BPC]} for k in range(N_CORES)]
    res = run_bass_kernel_spmd(nc, in_maps, list(range(N_CORES)))
    return np.concatenate([res.results[k]["out"] for k in range(N_CORES)], axis=0)
